# revision 20
# baseline (speedup 1.0000x reference)
"""Trainium2 Bass kernel for ArrangementsContinuousDecoderRNN.

Architecture (per example): text bi-GRU encoder (T=128, H=512), clip bi-GRU
encoder (L=24), additive attention (queries independent of decoder state),
decoder GRU over [clip_enc, attn], sigmoid xy head + log_softmax o head.

Strategy: pure data parallel across 8 NeuronCores (8 examples each), with the
whole network in a "transposed" layout: feature dim on SBUF partitions,
(batch, time) on the free dim.  This lets every GRU step run as
ghT[1536, B] = WhhT.T @ hT with the weight tiles stationary, and the gate
elementwise math on [128, 4*B] tiles.  Matmuls in bf16 (FWL weight loads),
gates in f32.  Embedding gathers + all weight layout prep happen on host.
"""

import functools
import numpy as np
import ml_dtypes

import concourse.bass as bass
import concourse.bacc as bacc
import concourse.mybir as mybir
from concourse import tile
from concourse.bass_utils import run_bass_kernel_spmd

BF16 = mybir.dt.bfloat16
F32 = mybir.dt.float32
AF = mybir.ActivationFunctionType
ALU = mybir.AluOpType
AX = mybir.AxisListType

P = 128
H = 512
HC = H // P            # 4 h-chunks
G3 = 3 * H             # 1536
T_FULL, L_FULL, B_FULL = 128, 24, 64
NCORES = 8
X_SCALE, Y_SCALE = 500.0, 400.0  # X_PAD-2, Y_PAD-2

nbf16 = ml_dtypes.bfloat16


# ---------------------------------------------------------------- builder ----

def _gru_chain(nc, tc, pools, dirs, S, B, _unused, zeros4):
    """Emit S sequential GRU steps for one or two directions.

    dirs: list of (w_hh_sb [P,HC,G3] bf16, xp_sb [P,12,B,S] f32,
                   bhn_sb [P,HC] f32 or None, enc [P,HC,B,S] bf16 out,
                   reverse: bool)
    """
    psum, gp = pools["psum_rec"], pools["gates"]
    for t in range(S):
        for di, (w_hh, xp, bhn, enc, rev) in enumerate(dirs):
            tt = (S - 1 - t) if rev else t
            tprev = (S - t) if rev else (t - 1)
            ps = []
            for g in range(3):
                pg = psum.tile([P, HC, B], F32, tag=f"g{g}d{di}")
                for j in range(HC):
                    for k in range(HC):
                        rhs = (zeros4[:, k, :] if t == 0
                               else enc[:, k, :, tprev])
                        nc.tensor.matmul(
                            pg[:, j, :],
                            w_hh[:, k, g * H + j * P: g * H + (j + 1) * P],
                            rhs, start=(k == 0), stop=(k == HC - 1))
                ps.append(pg)
            xr, xz, xn = (xp[:, 0:4, :, tt], xp[:, 4:8, :, tt],
                          xp[:, 8:12, :, tt])
            r_t = gp.tile([P, HC, B], F32, tag=f"r{di}")
            z_t = gp.tile([P, HC, B], F32, tag=f"z{di}")
            n_t = gp.tile([P, HC, B], F32, tag=f"n{di}")
            nc.vector.tensor_add(r_t[:], ps[0][:], xr)
            nc.scalar.activation(r_t[:], r_t[:], AF.Sigmoid)
            nc.vector.tensor_add(z_t[:], ps[1][:], xz)
            nc.scalar.activation(z_t[:], z_t[:], AF.Sigmoid)
            if bhn is not None:
                nc.vector.tensor_add(
                    n_t[:], ps[2][:],
                    bhn[:, :, None].to_broadcast([P, HC, B]))
                nc.vector.tensor_mul(n_t[:], n_t[:], r_t[:])
            else:
                nc.vector.tensor_mul(n_t[:], ps[2][:], r_t[:])
            nc.vector.tensor_add(n_t[:], n_t[:], xn)
            nc.scalar.activation(n_t[:], n_t[:], AF.Tanh)
            d_t = gp.tile([P, HC, B], F32, tag=f"d{di}")
            hprev = (zeros4[:] if t == 0 else enc[:, :, :, tprev])
            nc.vector.tensor_sub(d_t[:], hprev, n_t[:])
            nc.vector.tensor_mul(d_t[:], z_t[:], d_t[:])
            nc.vector.tensor_add(enc[:, :, :, tt], n_t[:], d_t[:])


def _in_proj(nc, tc, pools, w_ih_sb, KC, bias_sb, rhs_of_k, N_TOT, xp_sb, ones):
    """xp[P,12,B,S] (f32) = (W_ihT | bias).T @ (x | 1).  rhs_of_k(k, sl) gives
    the [P, n] moving slice for contraction chunk k and free slice sl."""
    psum = pools["psum_mm"]
    NS = max(1, N_TOT // 512)
    NCHUNK = N_TOT // NS
    for m in range(12):
        for ns in range(NS):
            sl = slice(ns * NCHUNK, (ns + 1) * NCHUNK)
            pm = psum.tile([P, NCHUNK], F32, tag="inproj")
            for k in range(KC):
                nc.tensor.matmul(pm[:], w_ih_sb[:, k, m * P:(m + 1) * P],
                                 rhs_of_k(k, sl), start=(k == 0), stop=False)
            nc.tensor.matmul(pm[:], bias_sb[0:1, m * P:(m + 1) * P],
                             ones[0:1, 0:NCHUNK], start=False, stop=True)
            # xp free layout is (B, S) flattened; write via flat view
            nc.vector.tensor_copy(xp_sb[:, m, sl], pm[:])


def build_nc(B=8, T=128, L=24, dbg=False, zero_bhn=False):
    nc = bacc.Bacc()
    BT, BL = B * T, B * L

    def din(name, shape, dt=BF16):
        return nc.declare_dram_parameter(name, list(shape), dt, isOutput=False)

    xt = din("xt", [H, B, T])
    xc = din("xc", [H, B, L])
    w = {}
    for d, kin in (("tf", H), ("tb", H), ("cf", H), ("cb", H), ("de", 4 * H)):
        w[d + "_ih"] = din(f"w_{d}_ih", [kin, G3])
        w[d + "_b"] = din(f"w_{d}_b", [1, G3])
        w[d + "_hh"] = din(f"w_{d}_hh", [H, G3])
        w[d + "_bhn"] = din(f"w_{d}_bhn", [P, HC], F32)
    wv = din("w_v", [2 * H, H]); wvb = din("w_v_b", [1, H])
    wh = din("w_h", [2 * H, H]); whb = din("w_h_b", [1, H])
    wu = din("w_u", [P, HC])
    wxy = din("w_xy", [H, 2]); wxyb = din("w_xy_b", [1, 2])
    wo = din("w_o", [H, 2]); wob = din("w_o_b", [1, 2])
    out = nc.declare_dram_parameter("out", [BL, 4], F32, isOutput=True)
    if dbg:
        dbg_enc = nc.declare_dram_parameter("dbg_enc", [P, 8, B, T], BF16, True)
        dbg_clip = nc.declare_dram_parameter("dbg_clip", [P, 8, B, L], BF16, True)
        dbg_z = nc.declare_dram_parameter("dbg_z", [P, 8, B, L], BF16, True)
        dbg_do = nc.declare_dram_parameter("dbg_do", [P, HC, B, L], BF16, True)
        dbg_xpf = nc.declare_dram_parameter("dbg_xpf", [P, 12, B, T], F32, True)

    def ldw(pool, dram, KC, n, dt=BF16):
        t = pool.tile([P, KC, n], dt, tag=f"w_{dram.name}")
        nc.sync.dma_start(t[:], dram[:].rearrange("(c p) n -> p c n", p=P))
        return t

    def ldrow(pool, dram, n):
        t = pool.tile([1, n], BF16, tag=f"r_{dram.name}")
        nc.sync.dma_start(t[:], dram[:])
        return t

    with tile.TileContext(nc) as tc:
        with tc.tile_pool(name="persist", bufs=1) as pp:
            enc_f = pp.tile([P, HC, B, T], BF16)
            enc_b = pp.tile([P, HC, B, T], BF16)
            clip_f = pp.tile([P, HC, B, L], BF16)
            clip_b = pp.tile([P, HC, B, L], BF16)
            zT = pp.tile([P, 8, B, L], BF16)
            douts = pp.tile([P, HC, B, L], BF16)
            ones = pp.tile([1, 512], BF16)
            nc.gpsimd.memset(ones[:], 1.0)
            zeros4 = pp.tile([P, HC, B], BF16)
            nc.gpsimd.memset(zeros4[:], 0.0)
            idb = pp.tile([P, P], BF16)
            from concourse.masks import make_identity
            make_identity(nc, idb[:])
            idf = pp.tile([P, P], F32)
            make_identity(nc, idf[:])

            # ---------------- text phase ----------------
            with tc.tile_pool(name="wtext", bufs=1) as wp, \
                 tc.tile_pool(name="xptext", bufs=1) as xpp, \
                 tc.tile_pool(name="ps_mm", bufs=2, space=bass.MemorySpace.PSUM) as psmm, \
                 tc.tile_pool(name="ps_rec", bufs=1, space=bass.MemorySpace.PSUM) as psrec, \
                 tc.tile_pool(name="gates", bufs=2) as gp:
                pools = {"psum_mm": psmm, "psum_rec": psrec, "gates": gp}
                xT = wp.tile([P, HC, B, T], BF16)
                nc.sync.dma_start(xT[:], xt[:].rearrange("(c p) b t -> p c b t", p=P))
                dirs = []
                for di, d in enumerate(("tf", "tb")):
                    wih = ldw(wp, w[d + "_ih"], HC, G3)
                    whh = ldw(wp, w[d + "_hh"], HC, G3)
                    bia = ldrow(wp, w[d + "_b"], G3)
                    if zero_bhn:
                        bhn = None
                    else:
                        bhn = wp.tile([P, HC], F32, tag=f"bhn_{d}")
                        nc.sync.dma_start(bhn[:], w[d + "_bhn"][:])
                    xp = xpp.tile([P, 12, BT], F32, tag=f"xp{di}")
                    xTf = xT[:].rearrange("p c b t -> p c (b t)")
                    _in_proj(nc, tc, pools, wih, HC, bia,
                             lambda k, sl: xTf[:, k, sl], BT, xp, ones)
                    dirs.append((whh, xp[:].rearrange("p m (b t) -> p m b t", b=B),
                                 bhn, enc_f if di == 0 else enc_b, di == 1))
                if dbg:
                    nc.sync.dma_start(dbg_xpf[:],
                                      dirs[0][1])
                _gru_chain(nc, tc, pools, dirs, T, B, None, zeros4)
            if dbg:
                nc.sync.dma_start(dbg_enc[:, 0:HC], enc_f[:])
                nc.sync.dma_start(dbg_enc[:, HC:8], enc_b[:])

            # ---------------- clip phase ----------------
            with tc.tile_pool(name="wclip", bufs=1) as wp, \
                 tc.tile_pool(name="xpclip", bufs=1) as xpp, \
                 tc.tile_pool(name="ps_mm", bufs=2, space=bass.MemorySpace.PSUM) as psmm, \
                 tc.tile_pool(name="ps_rec", bufs=1, space=bass.MemorySpace.PSUM) as psrec, \
                 tc.tile_pool(name="gates", bufs=2) as gp:
                pools = {"psum_mm": psmm, "psum_rec": psrec, "gates": gp}
                xcT = wp.tile([P, HC, B, L], BF16)
                nc.sync.dma_start(xcT[:], xc[:].rearrange("(c p) b t -> p c b t", p=P))
                dirs = []
                for di, d in enumerate(("cf", "cb")):
                    wih = ldw(wp, w[d + "_ih"], HC, G3)
                    whh = ldw(wp, w[d + "_hh"], HC, G3)
                    bia = ldrow(wp, w[d + "_b"], G3)
                    if zero_bhn:
                        bhn = None
                    else:
                        bhn = wp.tile([P, HC], F32, tag=f"bhn_{d}")
                        nc.sync.dma_start(bhn[:], w[d + "_bhn"][:])
                    xp = xpp.tile([P, 12, BL], F32, tag=f"xp{di}")
                    xcf = xcT[:].rearrange("p c b t -> p c (b t)")
                    _in_proj(nc, tc, pools, wih, HC, bia,
                             lambda k, sl: xcf[:, k, sl], BL, xp, ones)
                    dirs.append((whh, xp[:].rearrange("p m (b t) -> p m b t", b=B),
                                 bhn, clip_f if di == 0 else clip_b, di == 1))
                _gru_chain(nc, tc, pools, dirs, L, B, None, zeros4)
            if dbg:
                nc.sync.dma_start(dbg_clip[:, 0:HC], clip_f[:])
                nc.sync.dma_start(dbg_clip[:, HC:8], clip_b[:])

            # ---------------- attention phase ----------------
            with tc.tile_pool(name="wattn", bufs=1) as wp, \
                 tc.tile_pool(name="attn", bufs=1) as ap, \
                 tc.tile_pool(name="ps_mm", bufs=1, space=bass.MemorySpace.PSUM) as psmm, \
                 tc.tile_pool(name="ps_sc", bufs=1, space=bass.MemorySpace.PSUM) as pssc, \
                 tc.tile_pool(name="soft", bufs=3) as sp, \
                 tc.tile_pool(name="ftile", bufs=3) as fp:
                wvs = ldw(wp, wv, 8, H)
                wvbs = ldrow(wp, wvb, H)
                whs = ldw(wp, wh, 8, H)
                whbs = ldrow(wp, whb, H)
                wus = wp.tile([P, HC], BF16)
                nc.sync.dma_start(wus[:], wu[:])
                ov = ap.tile([P, HC, B, T], BF16)
                oh = ap.tile([P, HC, B, L], F32)
                encf_f = enc_f[:].rearrange("p c b t -> p c (b t)")
                encf_b = enc_b[:].rearrange("p c b t -> p c (b t)")
                encf = lambda k: encf_f[:, k] if k < HC else encf_b[:, k - HC]
                ovf = ov[:].rearrange("p c b t -> p c (b t)")
                clf_f = clip_f[:].rearrange("p c b t -> p c (b t)")
                clf_b = clip_b[:].rearrange("p c b t -> p c (b t)")
                clf = lambda k: clf_f[:, k] if k < HC else clf_b[:, k - HC]
                ohf = oh[:].rearrange("p c b t -> p c (b t)")
                NSV = max(1, BT // 512)
                NCV = BT // NSV
                for m in range(HC):
                    for ns in range(NSV):
                        sl = slice(ns * NCV, (ns + 1) * NCV)
                        pm = psmm.tile([P, NCV], F32, tag="ov")
                        for k in range(8):
                            nc.tensor.matmul(pm[:], wvs[:, k, m * P:(m + 1) * P],
                                             encf(k)[:, sl], start=(k == 0), stop=False)
                        nc.tensor.matmul(pm[:], wvbs[0:1, m * P:(m + 1) * P],
                                         ones[0:1, 0:NCV], start=False, stop=True)
                        nc.vector.tensor_copy(ovf[:, m, sl], pm[:])
                    pm = psmm.tile([P, BL], F32, tag="oh")
                    for k in range(8):
                        nc.tensor.matmul(pm[:], whs[:, k, m * P:(m + 1) * P],
                                         clf(k)[:, :], start=(k == 0), stop=False)
                    nc.tensor.matmul(pm[:], whbs[0:1, m * P:(m + 1) * P],
                                     ones[0:1, 0:BL], start=False, stop=True)
                    nc.vector.tensor_copy(ohf[:, m, :], pm[:])
                # text_enc in [t, d] layout per example (for z matmuls)
                ted = ap.tile([T, 8, B, P], BF16)
                for b in range(B):
                    for dc in range(8):
                        pt = pssc.tile([T, P], BF16, tag="tr")
                        src_enc = enc_f if dc < HC else enc_b
                        nc.tensor.transpose(pt[:], src_enc[:, dc % HC, b, :], idb[:])
                        nc.vector.tensor_copy(ted[:, dc, b, :], pt[:])
                for b in range(B):
                    flat = sp.tile([1, L, T], F32, tag="fl")
                    for l in range(L):
                        ft = fp.tile([P, HC, T], BF16, tag="f")
                        for c in range(HC):
                            nc.scalar.activation(ft[:, c, :], ov[:, c, b, :],
                                                 AF.Tanh, bias=oh[:, c, b, l:l + 1])
                        scp = pssc.tile([1, T], F32, tag="sc")
                        for c in range(HC):
                            nc.tensor.matmul(scp[0:1, :], wus[:, c:c + 1],
                                             ft[:, c, :], start=(c == 0),
                                             stop=(c == HC - 1))
                        nc.vector.tensor_copy(flat[0:1, l, :], scp[0:1, :])
                    sc = sp.tile([L, T], F32, tag="scb")
                    nc.sync.dma_start(sc[:], flat[0:1, :, :])
                    mx = sp.tile([L, 1], F32, tag="mx")
                    nc.vector.reduce_max(mx[:], sc[:], axis=AX.X, negate=True)
                    et = sp.tile([L, T], F32, tag="et")
                    nc.scalar.activation(et[:], sc[:], AF.Exp, bias=mx[:])
                    sm = sp.tile([L, 1], F32, tag="sm")
                    nc.vector.reduce_sum(sm[:], et[:], axis=AX.X)
                    nc.vector.reciprocal(sm[:], sm[:])
                    at = sp.tile([L, T], F32, tag="at")
                    nc.vector.tensor_scalar_mul(at[:], et[:], sm[:])
                    sm = None
                    pat = pssc.tile([T, L], F32, tag="pat")
                    nc.tensor.transpose(pat[:], at[:], idf[0:L, 0:L])
                    ats = sp.tile([T, L], BF16, tag="ats")
                    nc.vector.tensor_copy(ats[:], pat[:])
                    for dc in range(8):
                        pz = pssc.tile([P, L], F32, tag="pz")
                        nc.tensor.matmul(pz[:], ted[:, dc, b, :], ats[:],
                                         start=True, stop=True)
                        nc.vector.tensor_copy(zT[:, dc, b, :], pz[:])
            if dbg:
                nc.sync.dma_start(dbg_z[:], zT[:])

            # ---------------- decoder phase ----------------
            with tc.tile_pool(name="wdec", bufs=1) as wp, \
                 tc.tile_pool(name="xpdec", bufs=1) as xpp, \
                 tc.tile_pool(name="ps_mm", bufs=2, space=bass.MemorySpace.PSUM) as psmm, \
                 tc.tile_pool(name="ps_rec", bufs=2, space=bass.MemorySpace.PSUM) as psrec, \
                 tc.tile_pool(name="gates", bufs=2) as gp:
                pools = {"psum_mm": psmm, "psum_rec": psrec, "gates": gp}
                wih = ldw(wp, w["de_ih"], 16, G3)
                whh = ldw(wp, w["de_hh"], HC, G3)
                bia = ldrow(wp, w["de_b"], G3)
                if zero_bhn:
                    bhn = None
                else:
                    bhn = wp.tile([P, HC], F32)
                    nc.sync.dma_start(bhn[:], w["de_bhn"][:])
                xp = xpp.tile([P, 12, BL], F32)
                clf_f2 = clip_f[:].rearrange("p c b t -> p c (b t)")
                clf_b2 = clip_b[:].rearrange("p c b t -> p c (b t)")
                zf = zT[:].rearrange("p c b t -> p c (b t)")

                def dec_rhs(k, sl):
                    if k < HC:
                        return clf_f2[:, k, sl]
                    if k < 8:
                        return clf_b2[:, k - HC, sl]
                    return zf[:, k - 8, sl]
                _in_proj(nc, tc, pools, wih, 16, bia, dec_rhs, BL, xp, ones)
                _gru_chain(nc, tc, pools,
                           [(whh, xp[:].rearrange("p m (b t) -> p m b t", b=B),
                             bhn, douts, False)], L, B, None, zeros4)
            if dbg:
                nc.sync.dma_start(dbg_do[:], douts[:])

            # ---------------- heads ----------------
            with tc.tile_pool(name="whead", bufs=1) as wp, \
                 tc.tile_pool(name="head", bufs=2) as hp, \
                 tc.tile_pool(name="ps_hd", bufs=2, space=bass.MemorySpace.PSUM) as psh:
                wxys = ldw(wp, wxy, HC, 2)
                wxybs = ldrow(wp, wxyb, 2)
                wos = ldw(wp, wo, HC, 2)
                wobs = ldrow(wp, wob, 2)
                dflat = douts[:].rearrange("p c b t -> p c (b t)")
                MH = 96
                for half in range((BL + MH - 1) // MH):
                    sl = slice(half * MH, min((half + 1) * MH, BL))
                    n = sl.stop - sl.start
                    pxy = psh.tile([MH, 2], F32, tag="pxy")
                    po = psh.tile([MH, 2], F32, tag="po")
                    for k in range(HC):
                        nc.tensor.matmul(pxy[0:n, :], dflat[:, k, sl],
                                         wxys[:, k, :], start=(k == 0), stop=False)
                        nc.tensor.matmul(po[0:n, :], dflat[:, k, sl],
                                         wos[:, k, :], start=(k == 0), stop=False)
                    nc.tensor.matmul(pxy[0:n, :], ones[0:1, 0:n], wxybs[0:1, :],
                                     start=False, stop=True)
                    nc.tensor.matmul(po[0:n, :], ones[0:1, 0:n], wobs[0:1, :],
                                     start=False, stop=True)
                    res = hp.tile([MH, 4], F32, tag="res")
                    nc.scalar.activation(res[0:n, 0:2], pxy[0:n, :], AF.Sigmoid)
                    nc.vector.tensor_scalar_mul(res[0:n, 0:1], res[0:n, 0:1], X_SCALE)
                    nc.vector.tensor_scalar_mul(res[0:n, 1:2], res[0:n, 1:2], Y_SCALE)
                    lg = hp.tile([MH, 2], F32, tag="lg")
                    nc.vector.tensor_copy(lg[0:n, :], po[0:n, :])
                    nmx = hp.tile([MH, 1], F32, tag="nmx")
                    nc.vector.reduce_max(nmx[0:n, :], lg[0:n, :], axis=AX.X,
                                         negate=True)
                    e2 = hp.tile([MH, 2], F32, tag="e2")
                    nc.scalar.activation(e2[0:n, :], lg[0:n, :], AF.Exp,
                                         bias=nmx[0:n, :])
                    s2 = hp.tile([MH, 1], F32, tag="s2")
                    nc.vector.reduce_sum(s2[0:n, :], e2[0:n, :], axis=AX.X)
                    nc.scalar.activation(s2[0:n, :], s2[0:n, :], AF.Ln)
                    t1 = hp.tile([MH, 1], F32, tag="t1")
                    nc.vector.tensor_sub(t1[0:n, :], nmx[0:n, :], s2[0:n, :])
                    nc.vector.tensor_scalar_add(res[0:n, 2:4], lg[0:n, :],
                                                t1[0:n, :])
                    nc.sync.dma_start(out[sl, :], res[0:n, :])
    if not nc.is_finalized():
        nc.finalize()
    return nc


# ------------------------------------------------------------- host side ----

def _np(a):
    return np.asarray(a, dtype=np.float32)


def prep_weights(params):
    """Host-side weight layout prep shared by all cores (returns f32/bf16)."""
    m = {}

    def gru(d, p):
        wih = _np(p["Wih"])          # [3H, in]
        whh = _np(p["Whh"])          # [3H, H]
        bih = _np(p["bih"])          # [3H]
        bhh = _np(p["bhh"])          # [3H]
        bias = bih.copy()
        bias[: 2 * H] += bhh[: 2 * H]
        m[f"w_{d}_ih"] = wih.T.copy().astype(nbf16)
        m[f"w_{d}_b"] = bias[None, :].astype(nbf16)
        m[f"w_{d}_hh"] = whh.T.copy().astype(nbf16)
        m[f"w_{d}_bhn"] = bhh[2 * H:].reshape(HC, P).T.copy().astype(np.float32)

    gru("tf", params["text_f"]); gru("tb", params["text_b"])
    gru("cf", params["clip_f"]); gru("cb", params["clip_b"])
    gru("de", params["dec"])
    ap = params["attn"]
    m["w_v"] = _np(ap["Wv"]).T.copy().astype(nbf16)        # [2H, H]
    m["w_v_b"] = _np(ap["bv"])[None, :].astype(nbf16)
    m["w_h"] = _np(ap["Wh"]).T.copy().astype(nbf16)
    m["w_h_b"] = _np(ap["bh"])[None, :].astype(nbf16)
    m["w_u"] = _np(ap["Wu"])[0].reshape(HC, P).T.copy().astype(nbf16)
    m["w_xy"] = _np(params["xy_W"]).T.copy().astype(nbf16)  # [H, 2]
    m["w_xy_b"] = _np(params["xy_b"])[None, :].astype(nbf16)
    m["w_o"] = _np(params["o_W"]).T.copy().astype(nbf16)
    m["w_o_b"] = _np(params["o_b"])[None, :].astype(nbf16)
    return m


def prep_inputs(text_inds, clip_inds, params, B, ncores):
    """Gather embeddings on host, shard batch, return per-core input maps."""
    te = _np(params["text_emb"])
    ce = _np(params["clip_emb"])
    ti = np.asarray(text_inds)
    ci = np.asarray(clip_inds)
    wmap = prep_weights(params)
    xt_all = te[ti]                   # [B_FULL, T, H]
    xc_all = ce[ci]                   # [B_FULL, L, H]
    in_maps = []
    for c in range(ncores):
        slc = slice(c * B, (c + 1) * B)
        xt = np.ascontiguousarray(
            xt_all[slc].transpose(2, 0, 1)).astype(nbf16)   # [H, B, T]
        xc = np.ascontiguousarray(
            xc_all[slc].transpose(2, 0, 1)).astype(nbf16)   # [H, B, L]
        im = {"xt": xt, "xc": xc}
        im.update(wmap)
        in_maps.append(im)
    return in_maps


@functools.lru_cache(maxsize=2)
def _built_nc(B, T, L, dbg, zero_bhn):
    return build_nc(B=B, T=T, L=L, dbg=dbg, zero_bhn=zero_bhn)


def run_device(text_inds, clip_inds, params, dbg=False, trace=False):
    B = B_FULL // NCORES
    zero_bhn = all(
        not np.any(np.asarray(params[k]["bhh"])[2 * H:])
        for k in ("text_f", "text_b", "clip_f", "clip_b", "dec"))
    nc = _built_nc(B, T_FULL, L_FULL, dbg, zero_bhn)
    in_maps = prep_inputs(text_inds, clip_inds, params, B, NCORES)
    res = run_bass_kernel_spmd(nc, in_maps, list(range(NCORES)), trace=trace)
    outs = [np.asarray(r["out"], dtype=np.float32) for r in res.results]
    full = np.concatenate([o.reshape(B, L_FULL, 4) for o in outs], axis=0)
    x_outs = full[:, :, 0]
    y_outs = full[:, :, 1]
    o_outs = full[:, :, 2:4]
    return (x_outs, y_outs, o_outs), res


def kernel(text_inds, clip_inds, params):
    (x_outs, y_outs, o_outs), _ = run_device(text_inds, clip_inds, params)
    return (x_outs, y_outs, o_outs)


# revision 22
# speedup vs baseline: 1.0393x; 1.0393x over previous
"""Trainium2 Bass kernel for ArrangementsContinuousDecoderRNN.

Architecture (per example): text bi-GRU encoder (T=128, H=512), clip bi-GRU
encoder (L=24), additive attention (queries independent of decoder state),
decoder GRU over [clip_enc, attn], sigmoid xy head + log_softmax o head.

Strategy: pure data parallel across 8 NeuronCores (8 examples each), with the
whole network in a "transposed" layout: feature dim on SBUF partitions,
(batch, time) on the free dim.  This lets every GRU step run as
ghT[1536, B] = WhhT.T @ hT with the weight tiles stationary, and the gate
elementwise math on [128, 4*B] tiles.  Matmuls in bf16 (FWL weight loads),
gates in f32.  Embedding gathers + all weight layout prep happen on host.
"""

import functools
import numpy as np
import ml_dtypes

import concourse.bass as bass
import concourse.bacc as bacc
import concourse.mybir as mybir
from concourse import tile
from concourse.bass_utils import run_bass_kernel_spmd

BF16 = mybir.dt.bfloat16
F32 = mybir.dt.float32
FP8 = mybir.dt.float8e4
WHH_SCALE = 32.0
SCI = 1.0 / WHH_SCALE
AF = mybir.ActivationFunctionType
ALU = mybir.AluOpType
AX = mybir.AxisListType

P = 128
H = 512
HC = H // P            # 4 h-chunks
G3 = 3 * H             # 1536
T_FULL, L_FULL, B_FULL = 128, 24, 64
NCORES = 8
X_SCALE, Y_SCALE = 500.0, 400.0  # X_PAD-2, Y_PAD-2

nbf16 = ml_dtypes.bfloat16


# ---------------------------------------------------------------- builder ----

def _gru_chain(nc, tc, pools, dirs, S, B, _unused, zeros4):
    """Emit S sequential GRU steps for one or two directions.

    dirs: list of (w_hh_sb [P,HC,G3] bf16, xp_sb [P,12,B,S] f32,
                   bhn_sb [P,HC] f32 or None, enc [P,HC,B,S] bf16 out,
                   reverse: bool)
    """
    ps_rz, ps_n, gp = pools["psum_rz"], pools["psum_n"], pools["gates"]
    for t in range(S):
        for di, (w_hh, xp, bhn, enc, rev) in enumerate(dirs):
            tt = (S - 1 - t) if rev else t
            tprev = (S - t) if rev else (t - 1)
            pg_rz = ps_rz.tile([P, 2 * HC, B], F32, tag=f"rz{di}")
            pg_n = ps_n.tile([P, HC, B], F32, tag=f"n{di}")
            for g in range(3):
                for j in range(HC):
                    dst = (pg_rz[:, g * HC + j, :] if g < 2
                           else pg_n[:, j, :])
                    for k in range(HC):
                        rhs = (zeros4[:, k, :] if t == 0
                               else enc[:, k, :, tprev])
                        nc.tensor.matmul(
                            dst,
                            w_hh[:, k, g * H + j * P: g * H + (j + 1) * P],
                            rhs, start=(k == 0), stop=(k == HC - 1))
            xn = xp[:, 8:12, :, tt]
            rz_t = gp.tile([P, 2 * HC, B], F32, tag=f"rz{di}")
            n_t = gp.tile([P, HC, B], F32, tag=f"n{di}")
            nc.vector.tensor_add(rz_t[:], pg_rz[:], xp[:, 0:8, :, tt])
            nc.scalar.activation(rz_t[:], rz_t[:], AF.Sigmoid, scale=SCI)
            if bhn is not None:
                nc.vector.tensor_add(
                    n_t[:], pg_n[:],
                    bhn[:, :, None].to_broadcast([P, HC, B]))
                nc.vector.tensor_mul(n_t[:], n_t[:], rz_t[:, 0:HC])
            else:
                nc.vector.tensor_mul(n_t[:], pg_n[:], rz_t[:, 0:HC])
            nc.vector.tensor_add(n_t[:], n_t[:], xn)
            nc.scalar.activation(n_t[:], n_t[:], AF.Tanh, scale=SCI)
            d_t = gp.tile([P, HC, B], F32, tag=f"d{di}")
            hprev = (zeros4[:] if t == 0 else enc[:, :, :, tprev])
            nc.vector.tensor_sub(d_t[:], hprev, n_t[:])
            nc.vector.tensor_mul(d_t[:], rz_t[:, HC:2 * HC], d_t[:])
            nc.vector.tensor_add(enc[:, :, :, tt], n_t[:], d_t[:])


def _in_proj(nc, tc, pools, w_ih_sb, KC, bias_sb, rhs_of_k, N_TOT, xp_sb, ones):
    """xp[P,12,B,S] (f32) = (W_ihT | bias).T @ (x | 1).  rhs_of_k(k, sl) gives
    the [P, n] moving slice for contraction chunk k and free slice sl."""
    psum = pools["psum_mm"]
    NS = max(1, N_TOT // 512)
    NCHUNK = N_TOT // NS
    for m in range(12):
        for ns in range(NS):
            sl = slice(ns * NCHUNK, (ns + 1) * NCHUNK)
            pm = psum.tile([P, NCHUNK], F32, tag="inproj")
            for k in range(KC):
                nc.tensor.matmul(pm[:], w_ih_sb[:, k, m * P:(m + 1) * P],
                                 rhs_of_k(k, sl), start=(k == 0), stop=False)
            nc.tensor.matmul(pm[:], bias_sb[0:1, m * P:(m + 1) * P],
                             ones[0:1, 0:NCHUNK], start=False, stop=True)
            # xp free layout is (B, S) flattened; write via flat view
            nc.vector.tensor_copy(xp_sb[:, m, sl], pm[:])


def build_nc(B=8, T=128, L=24, dbg=False, zero_bhn=False):
    nc = bacc.Bacc()
    BT, BL = B * T, B * L

    def din(name, shape, dt=BF16):
        return nc.declare_dram_parameter(name, list(shape), dt, isOutput=False)

    xt = din("xt", [H, B, T])
    xc = din("xc", [H, B, L])
    w = {}
    for d, kin in (("tf", H), ("tb", H), ("cf", H), ("cb", H), ("de", 4 * H)):
        w[d + "_ih"] = din(f"w_{d}_ih", [kin, G3])
        w[d + "_b"] = din(f"w_{d}_b", [1, G3])
        w[d + "_hh"] = din(f"w_{d}_hh", [H, G3])
        w[d + "_bhn"] = din(f"w_{d}_bhn", [P, HC], F32)
    wv = din("w_v", [2 * H, H]); wvb = din("w_v_b", [1, H])
    wh = din("w_h", [2 * H, H]); whb = din("w_h_b", [1, H])
    wu = din("w_u", [P, HC])
    wxy = din("w_xy", [H, 2]); wxyb = din("w_xy_b", [1, 2])
    wo = din("w_o", [H, 2]); wob = din("w_o_b", [1, 2])
    out = nc.declare_dram_parameter("out", [BL, 4], F32, isOutput=True)
    if dbg:
        dbg_enc = nc.declare_dram_parameter("dbg_enc", [P, 8, B, T], BF16, True)
        dbg_clip = nc.declare_dram_parameter("dbg_clip", [P, 8, B, L], BF16, True)
        dbg_z = nc.declare_dram_parameter("dbg_z", [P, 8, B, L], BF16, True)
        dbg_do = nc.declare_dram_parameter("dbg_do", [P, HC, B, L], BF16, True)
        dbg_xpf = nc.declare_dram_parameter("dbg_xpf", [P, 12, B, T], F32, True)

    def ldw(pool, dram, KC, n, dt=BF16):
        t = pool.tile([P, KC, n], dt, tag=f"w_{dram.name}")
        nc.sync.dma_start(t[:], dram[:].rearrange("(c p) n -> p c n", p=P))
        return t

    def ldrow(pool, dram, n):
        t = pool.tile([1, n], BF16, tag=f"r_{dram.name}")
        nc.sync.dma_start(t[:], dram[:])
        return t

    with tile.TileContext(nc, pool_alloc_mode="queue") as tc:
        with tc.tile_pool(name="persist", bufs=1) as pp:
            enc_f = pp.tile([P, HC, B, T], BF16)
            enc_b = pp.tile([P, HC, B, T], BF16)
            clip_f = pp.tile([P, HC, B, L], BF16)
            clip_b = pp.tile([P, HC, B, L], BF16)
            zT = pp.tile([P, 8, B, L], BF16)
            douts = pp.tile([P, HC, B, L], BF16)
            ones = pp.tile([1, 512], BF16)
            nc.gpsimd.memset(ones[:], 1.0)
            zeros4 = pp.tile([P, HC, B], BF16)
            nc.gpsimd.memset(zeros4[:], 0.0)
            idb = pp.tile([P, P], BF16)
            from concourse.masks import make_identity
            make_identity(nc, idb[:])
            idf = pp.tile([P, P], F32)
            make_identity(nc, idf[:])

            # ---------------- text phase ----------------
            with tc.tile_pool(name="wtext", bufs=1) as wp, \
                 tc.tile_pool(name="xptext", bufs=1) as xpp, \
                 tc.tile_pool(name="ps_mm", bufs=2, space=bass.MemorySpace.PSUM) as psmm, \
                 tc.tile_pool(name="ps_rz", bufs=2, space=bass.MemorySpace.PSUM) as psrz, \
                 tc.tile_pool(name="ps_n", bufs=1, space=bass.MemorySpace.PSUM) as psn, \
                 tc.tile_pool(name="gates", bufs=2) as gp:
                pools = {"psum_mm": psmm, "psum_rz": psrz, "psum_n": psn,
                         "gates": gp}
                xT = wp.tile([P, HC, B, T], BF16)
                nc.sync.dma_start(xT[:], xt[:].rearrange("(c p) b t -> p c b t", p=P))
                dirs = []
                for di, d in enumerate(("tf", "tb")):
                    wih = ldw(wp, w[d + "_ih"], HC, G3)
                    whh = ldw(wp, w[d + "_hh"], HC, G3)
                    bia = ldrow(wp, w[d + "_b"], G3)
                    if zero_bhn:
                        bhn = None
                    else:
                        bhn = wp.tile([P, HC], F32, tag=f"bhn_{d}")
                        nc.sync.dma_start(bhn[:], w[d + "_bhn"][:])
                    xp = xpp.tile([P, 12, BT], F32, tag=f"xp{di}")
                    xTf = xT[:].rearrange("p c b t -> p c (b t)")
                    _in_proj(nc, tc, pools, wih, HC, bia,
                             lambda k, sl: xTf[:, k, sl], BT, xp, ones)
                    dirs.append((whh, xp[:].rearrange("p m (b t) -> p m b t", b=B),
                                 bhn, enc_f if di == 0 else enc_b, di == 1))
                if dbg:
                    nc.sync.dma_start(dbg_xpf[:],
                                      dirs[0][1])
                _gru_chain(nc, tc, pools, dirs, T, B, None, zeros4)
            if dbg:
                nc.sync.dma_start(dbg_enc[:, 0:HC], enc_f[:])
                nc.sync.dma_start(dbg_enc[:, HC:8], enc_b[:])

            # ---------------- clip phase ----------------
            with tc.tile_pool(name="wclip", bufs=1) as wp, \
                 tc.tile_pool(name="xpclip", bufs=1) as xpp, \
                 tc.tile_pool(name="ps_mm", bufs=2, space=bass.MemorySpace.PSUM) as psmm, \
                 tc.tile_pool(name="ps_rz", bufs=2, space=bass.MemorySpace.PSUM) as psrz, \
                 tc.tile_pool(name="ps_n", bufs=1, space=bass.MemorySpace.PSUM) as psn, \
                 tc.tile_pool(name="gates", bufs=2) as gp:
                pools = {"psum_mm": psmm, "psum_rz": psrz, "psum_n": psn,
                         "gates": gp}
                xcT = wp.tile([P, HC, B, L], BF16)
                nc.sync.dma_start(xcT[:], xc[:].rearrange("(c p) b t -> p c b t", p=P))
                dirs = []
                for di, d in enumerate(("cf", "cb")):
                    wih = ldw(wp, w[d + "_ih"], HC, G3)
                    whh = ldw(wp, w[d + "_hh"], HC, G3)
                    bia = ldrow(wp, w[d + "_b"], G3)
                    if zero_bhn:
                        bhn = None
                    else:
                        bhn = wp.tile([P, HC], F32, tag=f"bhn_{d}")
                        nc.sync.dma_start(bhn[:], w[d + "_bhn"][:])
                    xp = xpp.tile([P, 12, BL], F32, tag=f"xp{di}")
                    xcf = xcT[:].rearrange("p c b t -> p c (b t)")
                    _in_proj(nc, tc, pools, wih, HC, bia,
                             lambda k, sl: xcf[:, k, sl], BL, xp, ones)
                    dirs.append((whh, xp[:].rearrange("p m (b t) -> p m b t", b=B),
                                 bhn, clip_f if di == 0 else clip_b, di == 1))
                _gru_chain(nc, tc, pools, dirs, L, B, None, zeros4)
            if dbg:
                nc.sync.dma_start(dbg_clip[:, 0:HC], clip_f[:])
                nc.sync.dma_start(dbg_clip[:, HC:8], clip_b[:])

            # ---------------- attention phase ----------------
            with tc.tile_pool(name="wattn", bufs=1) as wp, \
                 tc.tile_pool(name="attn", bufs=1) as ap, \
                 tc.tile_pool(name="ps_mm", bufs=1, space=bass.MemorySpace.PSUM) as psmm, \
                 tc.tile_pool(name="ps_sc", bufs=1, space=bass.MemorySpace.PSUM) as pssc, \
                 tc.tile_pool(name="soft", bufs=3) as sp, \
                 tc.tile_pool(name="ftile", bufs=3) as fp:
                wvs = ldw(wp, wv, 8, H)
                wvbs = ldrow(wp, wvb, H)
                whs = ldw(wp, wh, 8, H)
                whbs = ldrow(wp, whb, H)
                wus = wp.tile([P, HC], BF16)
                nc.sync.dma_start(wus[:], wu[:])
                ov = ap.tile([P, HC, B, T], BF16)
                oh = ap.tile([P, HC, B, L], F32)
                encf_f = enc_f[:].rearrange("p c b t -> p c (b t)")
                encf_b = enc_b[:].rearrange("p c b t -> p c (b t)")
                encf = lambda k: encf_f[:, k] if k < HC else encf_b[:, k - HC]
                ovf = ov[:].rearrange("p c b t -> p c (b t)")
                clf_f = clip_f[:].rearrange("p c b t -> p c (b t)")
                clf_b = clip_b[:].rearrange("p c b t -> p c (b t)")
                clf = lambda k: clf_f[:, k] if k < HC else clf_b[:, k - HC]
                ohf = oh[:].rearrange("p c b t -> p c (b t)")
                NSV = max(1, BT // 512)
                NCV = BT // NSV
                for m in range(HC):
                    for ns in range(NSV):
                        sl = slice(ns * NCV, (ns + 1) * NCV)
                        pm = psmm.tile([P, NCV], F32, tag="ov")
                        for k in range(8):
                            nc.tensor.matmul(pm[:], wvs[:, k, m * P:(m + 1) * P],
                                             encf(k)[:, sl], start=(k == 0), stop=False)
                        nc.tensor.matmul(pm[:], wvbs[0:1, m * P:(m + 1) * P],
                                         ones[0:1, 0:NCV], start=False, stop=True)
                        nc.vector.tensor_copy(ovf[:, m, sl], pm[:])
                    pm = psmm.tile([P, BL], F32, tag="oh")
                    for k in range(8):
                        nc.tensor.matmul(pm[:], whs[:, k, m * P:(m + 1) * P],
                                         clf(k)[:, :], start=(k == 0), stop=False)
                    nc.tensor.matmul(pm[:], whbs[0:1, m * P:(m + 1) * P],
                                     ones[0:1, 0:BL], start=False, stop=True)
                    nc.vector.tensor_copy(ohf[:, m, :], pm[:])
                # text_enc in [t, d] layout per example (for z matmuls)
                ted = ap.tile([T, 8, B, P], BF16)
                for b in range(B):
                    for dc in range(8):
                        pt = pssc.tile([T, P], BF16, tag="tr")
                        src_enc = enc_f if dc < HC else enc_b
                        nc.tensor.transpose(pt[:], src_enc[:, dc % HC, b, :], idb[:])
                        nc.vector.tensor_copy(ted[:, dc, b, :], pt[:])
                for b in range(B):
                    flat = sp.tile([1, L, T], F32, tag="fl")
                    for l in range(L):
                        ft = fp.tile([P, HC, T], BF16, tag="f")
                        nc.vector.tensor_add(
                            ft[:], ov[:, :, b, :],
                            oh[:, :, b, l:l + 1].to_broadcast([P, HC, T]))
                        nc.scalar.activation(ft[:], ft[:], AF.Tanh)
                        scp = pssc.tile([1, T], F32, tag="sc")
                        for c in range(HC):
                            nc.tensor.matmul(scp[0:1, :], wus[:, c:c + 1],
                                             ft[:, c, :], start=(c == 0),
                                             stop=(c == HC - 1))
                        nc.vector.tensor_copy(flat[0:1, l, :], scp[0:1, :])
                    sc = sp.tile([L, T], F32, tag="scb")
                    nc.sync.dma_start(sc[:], flat[0:1, :, :])
                    mx = sp.tile([L, 1], F32, tag="mx")
                    nc.vector.reduce_max(mx[:], sc[:], axis=AX.X, negate=True)
                    et = sp.tile([L, T], F32, tag="et")
                    nc.scalar.activation(et[:], sc[:], AF.Exp, bias=mx[:])
                    sm = sp.tile([L, 1], F32, tag="sm")
                    nc.vector.reduce_sum(sm[:], et[:], axis=AX.X)
                    nc.vector.reciprocal(sm[:], sm[:])
                    at = sp.tile([L, T], F32, tag="at")
                    nc.vector.tensor_scalar_mul(at[:], et[:], sm[:])
                    sm = None
                    pat = pssc.tile([T, L], F32, tag="pat")
                    nc.tensor.transpose(pat[:], at[:], idf[0:L, 0:L])
                    ats = sp.tile([T, L], BF16, tag="ats")
                    nc.vector.tensor_copy(ats[:], pat[:])
                    for dc in range(8):
                        pz = pssc.tile([P, L], F32, tag="pz")
                        nc.tensor.matmul(pz[:], ted[:, dc, b, :], ats[:],
                                         start=True, stop=True)
                        nc.vector.tensor_copy(zT[:, dc, b, :], pz[:])
            if dbg:
                nc.sync.dma_start(dbg_z[:], zT[:])

            # ---------------- decoder phase ----------------
            with tc.tile_pool(name="wdec", bufs=1) as wp, \
                 tc.tile_pool(name="xpdec", bufs=1) as xpp, \
                 tc.tile_pool(name="ps_mm", bufs=2, space=bass.MemorySpace.PSUM) as psmm, \
                 tc.tile_pool(name="ps_rz", bufs=2, space=bass.MemorySpace.PSUM) as psrz, \
                 tc.tile_pool(name="ps_n", bufs=2, space=bass.MemorySpace.PSUM) as psn, \
                 tc.tile_pool(name="gates", bufs=2) as gp:
                pools = {"psum_mm": psmm, "psum_rz": psrz, "psum_n": psn,
                         "gates": gp}
                wih = ldw(wp, w["de_ih"], 16, G3)
                whh = ldw(wp, w["de_hh"], HC, G3)
                bia = ldrow(wp, w["de_b"], G3)
                if zero_bhn:
                    bhn = None
                else:
                    bhn = wp.tile([P, HC], F32)
                    nc.sync.dma_start(bhn[:], w["de_bhn"][:])
                xp = xpp.tile([P, 12, BL], F32)
                clf_f2 = clip_f[:].rearrange("p c b t -> p c (b t)")
                clf_b2 = clip_b[:].rearrange("p c b t -> p c (b t)")
                zf = zT[:].rearrange("p c b t -> p c (b t)")

                def dec_rhs(k, sl):
                    if k < HC:
                        return clf_f2[:, k, sl]
                    if k < 8:
                        return clf_b2[:, k - HC, sl]
                    return zf[:, k - 8, sl]
                _in_proj(nc, tc, pools, wih, 16, bia, dec_rhs, BL, xp, ones)
                _gru_chain(nc, tc, pools,
                           [(whh, xp[:].rearrange("p m (b t) -> p m b t", b=B),
                             bhn, douts, False)], L, B, None, zeros4)
            if dbg:
                nc.sync.dma_start(dbg_do[:], douts[:])

            # ---------------- heads ----------------
            with tc.tile_pool(name="whead", bufs=1) as wp, \
                 tc.tile_pool(name="head", bufs=2) as hp, \
                 tc.tile_pool(name="ps_hd", bufs=2, space=bass.MemorySpace.PSUM) as psh:
                wxys = ldw(wp, wxy, HC, 2)
                wxybs = ldrow(wp, wxyb, 2)
                wos = ldw(wp, wo, HC, 2)
                wobs = ldrow(wp, wob, 2)
                dflat = douts[:].rearrange("p c b t -> p c (b t)")
                MH = 96
                for half in range((BL + MH - 1) // MH):
                    sl = slice(half * MH, min((half + 1) * MH, BL))
                    n = sl.stop - sl.start
                    pxy = psh.tile([MH, 2], F32, tag="pxy")
                    po = psh.tile([MH, 2], F32, tag="po")
                    for k in range(HC):
                        nc.tensor.matmul(pxy[0:n, :], dflat[:, k, sl],
                                         wxys[:, k, :], start=(k == 0), stop=False)
                        nc.tensor.matmul(po[0:n, :], dflat[:, k, sl],
                                         wos[:, k, :], start=(k == 0), stop=False)
                    nc.tensor.matmul(pxy[0:n, :], ones[0:1, 0:n], wxybs[0:1, :],
                                     start=False, stop=True)
                    nc.tensor.matmul(po[0:n, :], ones[0:1, 0:n], wobs[0:1, :],
                                     start=False, stop=True)
                    res = hp.tile([MH, 4], F32, tag="res")
                    nc.scalar.activation(res[0:n, 0:2], pxy[0:n, :], AF.Sigmoid)
                    nc.vector.tensor_scalar_mul(res[0:n, 0:1], res[0:n, 0:1], X_SCALE)
                    nc.vector.tensor_scalar_mul(res[0:n, 1:2], res[0:n, 1:2], Y_SCALE)
                    lg = hp.tile([MH, 2], F32, tag="lg")
                    nc.vector.tensor_copy(lg[0:n, :], po[0:n, :])
                    nmx = hp.tile([MH, 1], F32, tag="nmx")
                    nc.vector.reduce_max(nmx[0:n, :], lg[0:n, :], axis=AX.X,
                                         negate=True)
                    e2 = hp.tile([MH, 2], F32, tag="e2")
                    nc.scalar.activation(e2[0:n, :], lg[0:n, :], AF.Exp,
                                         bias=nmx[0:n, :])
                    s2 = hp.tile([MH, 1], F32, tag="s2")
                    nc.vector.reduce_sum(s2[0:n, :], e2[0:n, :], axis=AX.X)
                    nc.scalar.activation(s2[0:n, :], s2[0:n, :], AF.Ln)
                    t1 = hp.tile([MH, 1], F32, tag="t1")
                    nc.vector.tensor_sub(t1[0:n, :], nmx[0:n, :], s2[0:n, :])
                    nc.vector.tensor_scalar_add(res[0:n, 2:4], lg[0:n, :],
                                                t1[0:n, :])
                    nc.sync.dma_start(out[sl, :], res[0:n, :])
    if not nc.is_finalized():
        nc.finalize()
    return nc


# ------------------------------------------------------------- host side ----

def _np(a):
    return np.asarray(a, dtype=np.float32)


def prep_weights(params):
    """Host-side weight layout prep shared by all cores (returns f32/bf16)."""
    m = {}

    def gru(d, p):
        wih = _np(p["Wih"])          # [3H, in]
        whh = _np(p["Whh"])          # [3H, H]
        bih = _np(p["bih"])          # [3H]
        bhh = _np(p["bhh"])          # [3H]
        bias = bih.copy()
        bias[: 2 * H] += bhh[: 2 * H]
        m[f"w_{d}_ih"] = (wih.T * WHH_SCALE).copy().astype(nbf16)
        m[f"w_{d}_b"] = (bias[None, :] * WHH_SCALE).astype(nbf16)
        m[f"w_{d}_hh"] = (whh.T * WHH_SCALE).copy().astype(nbf16)
        m[f"w_{d}_bhn"] = (bhh[2 * H:] * WHH_SCALE).reshape(
            HC, P).T.copy().astype(np.float32)

    gru("tf", params["text_f"]); gru("tb", params["text_b"])
    gru("cf", params["clip_f"]); gru("cb", params["clip_b"])
    gru("de", params["dec"])
    ap = params["attn"]
    m["w_v"] = _np(ap["Wv"]).T.copy().astype(nbf16)        # [2H, H]
    m["w_v_b"] = _np(ap["bv"])[None, :].astype(nbf16)
    m["w_h"] = _np(ap["Wh"]).T.copy().astype(nbf16)
    m["w_h_b"] = _np(ap["bh"])[None, :].astype(nbf16)
    m["w_u"] = _np(ap["Wu"])[0].reshape(HC, P).T.copy().astype(nbf16)
    m["w_xy"] = _np(params["xy_W"]).T.copy().astype(nbf16)  # [H, 2]
    m["w_xy_b"] = _np(params["xy_b"])[None, :].astype(nbf16)
    m["w_o"] = _np(params["o_W"]).T.copy().astype(nbf16)
    m["w_o_b"] = _np(params["o_b"])[None, :].astype(nbf16)
    return m


def prep_inputs(text_inds, clip_inds, params, B, ncores):
    """Gather embeddings on host, shard batch, return per-core input maps."""
    te = _np(params["text_emb"])
    ce = _np(params["clip_emb"])
    ti = np.asarray(text_inds)
    ci = np.asarray(clip_inds)
    wmap = prep_weights(params)
    xt_all = te[ti]                   # [B_FULL, T, H]
    xc_all = ce[ci]                   # [B_FULL, L, H]
    in_maps = []
    for c in range(ncores):
        slc = slice(c * B, (c + 1) * B)
        xt = np.ascontiguousarray(
            xt_all[slc].transpose(2, 0, 1)).astype(nbf16)   # [H, B, T]
        xc = np.ascontiguousarray(
            xc_all[slc].transpose(2, 0, 1)).astype(nbf16)   # [H, B, L]
        im = {"xt": xt, "xc": xc}
        im.update(wmap)
        in_maps.append(im)
    return in_maps


@functools.lru_cache(maxsize=2)
def _built_nc(B, T, L, dbg, zero_bhn):
    return build_nc(B=B, T=T, L=L, dbg=dbg, zero_bhn=zero_bhn)


def run_device(text_inds, clip_inds, params, dbg=False, trace=False):
    B = B_FULL // NCORES
    zero_bhn = all(
        not np.any(np.asarray(params[k]["bhh"])[2 * H:])
        for k in ("text_f", "text_b", "clip_f", "clip_b", "dec"))
    nc = _built_nc(B, T_FULL, L_FULL, dbg, zero_bhn)
    in_maps = prep_inputs(text_inds, clip_inds, params, B, NCORES)
    res = run_bass_kernel_spmd(nc, in_maps, list(range(NCORES)), trace=trace)
    outs = [np.asarray(r["out"], dtype=np.float32) for r in res.results]
    full = np.concatenate([o.reshape(B, L_FULL, 4) for o in outs], axis=0)
    x_outs = full[:, :, 0]
    y_outs = full[:, :, 1]
    o_outs = full[:, :, 2:4]
    return (x_outs, y_outs, o_outs), res


def kernel(text_inds, clip_inds, params):
    (x_outs, y_outs, o_outs), _ = run_device(text_inds, clip_inds, params)
    return (x_outs, y_outs, o_outs)
